# revision 1
# baseline (speedup 1.0000x reference)
"""Trainium2 Bass kernel for fp8-quantized dense matmul (dense_mlp).

Reference computation (per-tensor dynamic fp8 e4m3fn quantization):
    x:     [8, 8192, 512] f32  -> x2d [M=65536, K=512]
    w:     [512, 512] f32
    xs     = 448 / max(amax(|x|), 1e-12);  x_q = e4m3fn(x * xs)
    ws     = 448 / max(amax(|w|), 1e-12);  w_q = e4m3fn(w * ws)
    out    = (x_q @ w_q) * (1/xs) * (1/ws)          [M, 512] f32

Sharding: data-parallel over M across 8 cores (8192 rows each), weight
replicated; the x amax needs a cross-core AllReduce(max).

TRN2 fp8e4 (float8_e4m3) maxes out at +-240 (values in (240, 448] that OCP
e4m3fn can represent are Inf/NaN on TRN). We therefore quantize on-device
with scale' = 224/amax = (448/amax)/2. Scaling by an exact power of two
keeps every quantized value on the same relative grid (q' = q/2 exactly,
modulo the subnormal tail which is negligible), and the dequant factor
computed from the halved scales is exactly 4x the reference's factor,
cancelling the psum/4 -- so the result matches the reference bit-for-bit
up to f32 summation order (HW rel err 4e-7 in Normal matmul mode; the
default DoubleRow fp8 perf mode measures ~1e-4 from the PE's paired-
product accumulation precision, and is ~16% faster end-to-end).

Performance structure (measured on HW via repeat-loop slopes):
  phase A  ~60us: the 16.78MB x load is HBM-bound at ~278 GB/s/core (the
           practical ceiling with all 8 cores streaming; chunk size and
           DGE path don't move it). amax (DVE), f32 PE transposes and ACT
           psum->sbuf evac all hide under the load.
  AllReduce(max) of the x amax: ~10us floor, excluded from the loop
           metric and added as a constant; the weight load+quantize hides
           in this window.
  phase B  ~38us: PE-bound. Per DoubleRow matmul = LDWEIGHTS(~220cyc) +
           512cyc streaming; 128 MMs. Dequant (ACT, psum->fp16) and DVE
           quantize hide under PE; fp16 stores (rel err 2.3e-4, 87x
           inside the 2e-2 budget) fully hide -- f32 stores did not,
           which is the main win over the f32-out baseline (-10.5us).
Weight-stationary variants (3 layouts) measured 20+us SLOWER despite
stationary reuse; DoubleRowSwInterleave gives wrong results with the
standard AP layout. The output is produced m-major so no host transpose
is needed; the host casts fp16 -> f32 on reassembly.
"""

from contextlib import nullcontext

import numpy as np

import concourse.bacc as bacc
import concourse.bass_isa as bass_isa
import concourse.mybir as mybir
import concourse.tile as tile
from concourse.bass_utils import run_bass_kernel_spmd
from concourse.masks import make_identity

F32 = mybir.dt.float32
F16 = mybir.dt.float16
FP8 = mybir.dt.float8e4

K = 512
N = 512
KB = K // 128  # k-blocks of 128 (partition-dim contraction tiles)
N_CORES = 8

# fp8 scale ceiling on TRN (e4m3 max normal is 240; 224 = 448/2 keeps the
# quantization grid exactly aligned with the reference's e4m3fn grid)
FP8_CEIL = 224.0


def build_nc(m_shard: int, n_cores: int = N_CORES, use_doublerow: bool = True,
             dma_chunk: int = 4, store_chunk: int = 2, repeat: int = 1,
             phase_a_only: bool = False, ostage_bufs: int | None = None,
             out_f16: bool = True, deq_dve_mod: int = 0, wstat: bool = False,
             empty_loop: bool = False, pa_mode: str = "full",
             pb_mode: str = "full", split_load: bool = False,
             swi: bool = False, load_pair: bool = False,
             kb_outer: bool = False, xld_bufs: int | None = None,
             small_xt: bool = False, swdge_load: bool = False,
             tr_f32r: bool = False, fast_par: bool = False,
             q_split: bool = True, pe_warm: int = 6):
    """Build + compile the per-core SPMD program.

    m_shard: rows of x handled by this core (must be divisible by 128*dma_chunk)
    repeat: >1 builds a TIMING variant -- the x pipeline (phases A+B and the
        scale chain, minus the AllReduce, which cannot sit inside control
        flow) runs in a hardware For_i loop `repeat` times so per-iteration
        time can be resolved above the ~0.5ms axon dispatch noise.
    """
    MT = m_shard // 128          # number of 128-row m-tiles
    CH = MT // dma_chunk         # number of DMA chunks
    SC = MT // store_chunk       # number of store chunks

    nc = bacc.Bacc(
        trn_type="TRN2",
        target_bir_lowering=False,
        debug=False,
        num_devices=n_cores,
    )

    out_dt = F16 if out_f16 else F32
    x_in = nc.dram_tensor("x", [m_shard, K], F32, kind="ExternalInput")
    w_in = nc.dram_tensor("w", [K, N], F32, kind="ExternalInput")
    # wstat: weight-stationary matmul writes the output n-major ([N, m]);
    # the host transposes back when reassembling shards
    out_shape = [N, m_shard] if wstat else [m_shard, N]
    out_d = nc.dram_tensor("out", out_shape, out_dt, kind="ExternalOutput")

    # DRAM views:
    #  x rows (c*dma_chunk + j)*128 + p  ->  [c, p, j, k]
    if load_pair:
        # paired-row layout: partition p takes rows {.. + 2p, .. + 2p+1} so
        # each DRAM run is 4KB (2 consecutive rows) instead of 2KB; the m
        # permutation is absorbed by the load and store views
        assert store_chunk == 2
        x_re = x_in.ap().rearrange("(c j p r) k -> c p j r k",
                                   j=dma_chunk // 2, p=128, r=2)
    else:
        x_re = x_in.ap().rearrange("(c j p) k -> c p j k", j=dma_chunk, p=128)
    #  w rows kb*128 + p -> [p, kb, n]
    w_re = w_in.ap().rearrange("(kb p) n -> p kb n", p=128)
    if wstat:
        # out_T[nt*128+p, mg*1024 + mm] -> [mg, p, nt, mm] (2KB runs per row)
        out_re = out_d.ap().rearrange("(nt p) (mg mm) -> mg p nt mm",
                                      p=128, mm=1024)
    elif load_pair:
        # row = c*256 + 2p + r; each partition writes 2 consecutive rows
        out_re = out_d.ap().rearrange("(c p r) n -> c p r n", p=128, r=2)
    else:
        out_re = out_d.ap().rearrange("(c j p) n -> c p j n", j=store_chunk, p=128)

    with tile.TileContext(nc) as tc:
        with (
            tc.tile_pool(name="pers", bufs=1) as pers,
            tc.tile_pool(name="xld",
                         bufs=xld_bufs if xld_bufs is not None
                         else (3 if wstat else max(2, 16 // dma_chunk))) as xld,
            tc.tile_pool(name="xqp", bufs=4 if wstat else 8) as xqp,
            tc.tile_pool(
                name="ostage",
                bufs=ostage_bufs if ostage_bufs is not None
                else (3 if wstat else (4 if store_chunk <= 2 else 3)),
            ) as ostage,
            tc.tile_pool(name="tpsum", bufs=2, space="PSUM") as tpsum,
            tc.tile_pool(name="opsum", bufs=2, space="PSUM") as opsum,
            tc.tile_pool(name="ccdram", bufs=1, space="DRAM") as ccdram,
        ):
            # ---------------- persistent tiles ----------------
            ident = pers.tile([128, 128], F32)
            w_f32 = pers.tile([128, KB, N], F32)
            wq = pers.tile([128, KB, N], FP8)
            # transposed x (K on partitions); probes shrink it to fit SBUF
            xt_f32 = pers.tile([128, KB, 128 if small_xt else m_shard], F32)
            amax_slots = pers.tile([128, CH - 1 + dma_chunk], F32)

            def sc(name):
                return pers.tile([128, 1], F32, name=name)

            wa_part, wa_bc, wa_c, wa_r = sc("wa_part"), sc("wa_bc"), sc("wa_c"), sc("wa_r")
            xa_part, xa_bc, xa_g, xa_c, xa_r = (
                sc("xa_part"), sc("xa_bc"), sc("xa_g"), sc("xa_c"), sc("xa_r"))
            xsc, inv_xsc, dsc = sc("xsc"), sc("inv_xsc"), sc("dsc")

            make_identity(nc, ident)

            ones_row = pers.tile([1, 128], F32, name="ones_row")
            xa_s = pers.tile([1, 1], F32, name="xa_s")
            if fast_par:
                nc.vector.memset(ones_row, 1.0)

            wpair = pers.tile([1, 2], F32, name="wpair")
            wsc_b = pers.tile([128, 2], F32, name="wsc_b")  # [wsc, 1/wsc] bcast

            def weight_path():
                # Quantize the (replicated) weight -- no collective needed.
                # Deliberately gpsimd-free: in the single-shot build this runs
                # during the x-amax AllReduce, and anything on gpsimd would
                # queue behind the collective's ~10us engine wait. The
                # partition reduce/broadcast go through PE instead.
                nc.sync.dma_start(out=w_f32[:], in_=w_re)
                nc.vector.tensor_reduce(
                    out=wa_part[:], in_=w_f32[:], axis=mybir.AxisListType.XY,
                    op=mybir.AluOpType.max, apply_absolute_value=True,
                )
                wa_t = tpsum.tile([1, 128], F32, name="wa_t", tag="tp")
                nc.tensor.transpose(wa_t[:], wa_part[:], ident[:])
                nc.vector.tensor_reduce(
                    out=wa_bc[0:1, :], in_=wa_t[:], axis=mybir.AxisListType.X,
                    op=mybir.AluOpType.max,
                )
                nc.vector.tensor_scalar_max(wa_c[0:1, :], wa_bc[0:1, :], 1e-12)
                # wsc = 224 * (1/wa)  (TT divide is not a valid TRN2 DVE op;
                # the extra rounding vs fl(224/wa) is <=1ulp on the scale)
                nc.vector.reciprocal(wa_r[0:1, :], wa_c[0:1, :])
                nc.vector.tensor_scalar_mul(wpair[:, 0:1], wa_r[0:1, :], FP8_CEIL)
                nc.vector.reciprocal(wpair[:, 1:2], wpair[:, 0:1])
                # broadcast [wsc, 1/wsc] to all 128 partitions: bounce the
                # 8B pair through DRAM, then re-read with a 0-stride
                # partition dim (exact; a PE-matmul broadcast would truncate
                # the scale to fp22; SBUF sources can't have 0-stride
                # partitions, DRAM sources can)
                wdram = ccdram.tile([1, 2], F32, name="wdram")
                nc.sync.dma_start(out=wdram[:], in_=wpair[:])
                nc.sync.dma_start(
                    out=wsc_b[:].rearrange("p (a b) -> p a b", a=1),
                    in_=wdram[:].partition_broadcast(128),
                )
                # quantize weight: wq = fp8(w * wsc)
                nc.scalar.mul(wq[:], w_f32[:], wsc_b[:, 0:1])

            # In timing builds the collective runs once, outside the loop
            # (collectives cannot appear inside control flow).
            timing_loop = repeat > 1
            if timing_loop:
                # w path cannot sit inside the loop (it must run once), and
                # instructions emitted after a For_i cannot execute within it
                weight_path()
                weight_path = None
            if timing_loop and n_cores > 1:
                nc.vector.memset(xa_bc, 1.0)
                cc_in0 = ccdram.tile([128, 1], F32)
                cc_out0 = ccdram.tile([128, 1], F32)
                nc.gpsimd.dma_start(out=cc_in0[:], in_=xa_bc[:])
                nc.gpsimd.collective_compute(
                    "AllReduce",
                    mybir.AluOpType.max,
                    replica_groups=[list(range(n_cores))],
                    ins=[cc_in0.opt()],
                    outs=[cc_out0.opt()],
                )
                nc.gpsimd.dma_start(out=xa_g[:], in_=cc_out0[:])

            loop_cm = (
                tc.For_i(0, repeat, 1, hint_engines=(mybir.EngineType.PE,))
                if timing_loop else nullcontext()
            )
            with loop_cm:
                x_pipeline(
                    nc, tc, CH, dma_chunk, SC, store_chunk, KB, MT, N,
                    x_re, out_re, xld, xqp, ostage, tpsum, opsum, ccdram,
                    ident, w_f32, wq, xt_f32, amax_slots,
                    xa_part, xa_bc, xa_g, xa_c, xa_r, xsc, inv_xsc, wsc_b, dsc,
                    n_cores, use_doublerow, use_collective=not timing_loop,
                    phase_a_only=phase_a_only, weight_path=weight_path,
                    out_dt=out_dt, deq_dve_mod=deq_dve_mod, wstat=wstat,
                    pa_mode=pa_mode, pb_mode=pb_mode, split_load=split_load,
                    empty_loop=empty_loop, swi=swi, load_pair=load_pair,
                    kb_outer=kb_outer, swdge_load=swdge_load, tr_f32r=tr_f32r,
                    fast_par=fast_par, ones_row=ones_row, xa_s=xa_s,
                    q_split=q_split, pe_warm=pe_warm,
                )

    nc.compile()
    return nc


def x_pipeline(nc, tc, CH, dma_chunk, SC, store_chunk, KB, MT, N,
               x_re, out_re, xld, xqp, ostage, tpsum, opsum, ccdram,
               ident, w_f32, wq, xt_f32, amax_slots,
               xa_part, xa_bc, xa_g, xa_c, xa_r, xsc, inv_xsc, wsc_b, dsc,
               n_cores, use_doublerow, use_collective, phase_a_only=False,
               weight_path=None, out_dt=F32, deq_dve_mod=0, wstat=False,
               pa_mode="full", pb_mode="full", split_load=False,
               empty_loop=False, swi=False, load_pair=False, kb_outer=False,
               swdge_load=False, tr_f32r=False, fast_par=False, ones_row=None,
               xa_s=None, q_split=False, pe_warm=0):
            if empty_loop:
                nc.vector.memset(xa_part, 0.0)
                return
            # ---------------- phase A: load x, amax, transpose ----------------
            load_eng = (lambda c: nc.gpsimd if swdge_load
                        else (nc.scalar if (split_load and c % 2) else nc.sync))

            def load_dma(eng, xt, c):
                if load_pair:
                    eng.dma_start(
                        out=xt[:].rearrange("p (j r) k -> p j r k", r=2),
                        in_=x_re[c])
                else:
                    eng.dma_start(out=xt[:], in_=x_re[c])

            def load_view_j(c, j):
                return (x_re[c, :, j // 2, j % 2, :] if load_pair
                        else x_re[c, :, j, :])

            for c in range(CH):
                xt = xld.tile([128, dma_chunk, K], F32)
                if c < CH - 1 or pa_mode == "load":
                    load_dma(load_eng(c), xt, c)
                    if pa_mode != "load":
                        nc.vector.tensor_reduce(
                            out=amax_slots[:, c:c + 1], in_=xt[:],
                            axis=mybir.AxisListType.XY,
                            op=mybir.AluOpType.max, apply_absolute_value=True,
                        )
                else:
                    # split the last chunk into per-m-tile DMAs + small amax
                    # ops so the final abs-max lands right after the final
                    # (small) load instead of one 2.2us op after a 1MiB one
                    for j in range(dma_chunk):
                        load_eng(c).dma_start(out=xt[:, j, :], in_=load_view_j(c, j))
                        nc.vector.tensor_reduce(
                            out=amax_slots[:, c + j:c + j + 1], in_=xt[:, j, :],
                            axis=mybir.AxisListType.X,
                            op=mybir.AluOpType.max, apply_absolute_value=True,
                        )
                if pa_mode in ("load", "amax"):
                    continue
                for j2 in range(dma_chunk // 2):
                    # two m-tiles per PSUM tile (2 banks) -> one FD-1024 evac
                    tp = tpsum.tile([128, 2, KB, 128], F32)
                    for j in (2 * j2, 2 * j2 + 1):
                        for kb in range(KB):
                            if tr_f32r:
                                # f32r transpose runs 1.5 cyc/row vs 2.0 for
                                # f32; pure data movement, bit-exact
                                F32R = mybir.dt.float32r
                                nc.tensor.transpose(
                                    tp[:, j % 2, kb, :].bitcast(F32R),
                                    xt[:, j, kb * 128:(kb + 1) * 128]
                                    .bitcast(F32R),
                                    ident[:].bitcast(F32R),
                                )
                            else:
                                nc.tensor.transpose(
                                    tp[:, j % 2, kb, :],
                                    xt[:, j, kb * 128:(kb + 1) * 128], ident[:],
                                )
                    i = c * dma_chunk + 2 * j2   # first of the 2 m-tiles
                    # evacuate transposed f32 tiles (ACT; PSUM -> SBUF)
                    # dest [128, kb, 2, 128] viewed per kb: [2 m-tiles, 128]
                    nc.scalar.copy(
                        out=xt_f32[:, :, i * 128:(i + 2) * 128]
                        .rearrange("p kb (j m) -> p j kb m", j=2),
                        in_=tp[:],
                    )

            if phase_a_only:
                return

            # ---------------- amax finalize + collective ----------------
            # Emission order matters: engine streams execute IN ORDER, so the
            # x-amax chain (which gates everything) is emitted before the
            # weight path on every engine it touches.
            nc.vector.tensor_reduce(
                out=xa_part[:], in_=amax_slots[:], axis=mybir.AxisListType.X,
                op=mybir.AluOpType.max,
            )
            if pe_warm:
                # keep the PE array busy (p-state) across the amax->scale
                # gap: dummy transposes gated on the last-loaded chunk, sized
                # to finish before the first quantize op can complete
                for wi in range(pe_warm):
                    tpw = tpsum.tile([128, 128], F32, name=f"pe_warm{wi}",
                                     tag="tp")
                    nc.tensor.transpose(
                        tpw[:], xt[:, dma_chunk - 1, 0:128], ident[:])
            if fast_par:
                # partition max + broadcast without gpsimd: PE transpose ->
                # DVE reduce (to SBUF) -> PE ones-matmul broadcast -> ACT evac
                xa_t = tpsum.tile([1, 128], F32, name="xa_t", tag="tp")
                nc.tensor.transpose(xa_t[:], xa_part[:], ident[:])
                nc.vector.tensor_reduce(
                    out=xa_s[:], in_=xa_t[:], axis=mybir.AxisListType.X,
                    op=mybir.AluOpType.max,
                )
                bc = tpsum.tile([128, 1], F32, name="xa_bc_ps", tag="tp")
                nc.tensor.matmul(bc[:], ones_row[0:1, :], xa_s[:],
                                 start=True, stop=True)
                nc.scalar.copy(xa_bc[:], bc[:])
            else:
                nc.gpsimd.partition_all_reduce(
                    xa_bc[:], xa_part[:], channels=128,
                    reduce_op=bass_isa.ReduceOp.max,
                )
            cc_in = cc_out = None
            if use_collective and n_cores > 1:
                cc_in = ccdram.tile([128, 1], F32)
                cc_out = ccdram.tile([128, 1], F32)
                # bounces on HWDGE (nc.sync, ~0.6us first-byte vs SWDGE
                # ~1-2us); SP-queue order stays hazard-free: loads ->
                # bounce-out -> w path -> bounce-in -> stores
                nc.sync.dma_start(out=cc_in[:], in_=xa_bc[:])
                nc.gpsimd.collective_compute(
                    "AllReduce",
                    mybir.AluOpType.max,
                    replica_groups=[list(range(n_cores))],
                    ins=[cc_in.opt()],
                    outs=[cc_out.opt()],
                )

            if weight_path is not None:
                # runs during the collective: the 1MB weight DMA + wq chain
                # fill the DMA/DVE/ACT gap instead of competing with phase A
                weight_path()

            if cc_out is not None:
                nc.sync.dma_start(out=xa_g[:], in_=cc_out[:])
            else:
                nc.vector.tensor_copy(xa_g[:], xa_bc[:])

            nc.vector.tensor_scalar_max(xa_c[:], xa_g[:], 1e-12)
            nc.vector.reciprocal(xa_r[:], xa_c[:])
            nc.vector.tensor_scalar_mul(xsc[:], xa_r[:], FP8_CEIL)
            def emit_dsc():
                # emitted after the first quantize: DVE executes in order, so
                # placing these two ops between xsc and quantize_0 would delay
                # the first matmul/store; dsc is only needed by the first
                # dequant, which waits on the matmuls anyway
                nc.vector.reciprocal(inv_xsc[:], xsc[:])
                nc.vector.tensor_tensor(
                    out=dsc[:], in0=inv_xsc[:], in1=wsc_b[:, 1:2],
                    op=mybir.AluOpType.mult,
                )

            # ---------------- phase B: quantize, matmul, dequant, store -------
            if wstat == 2:
                # weight-stationary, take 2: quantize repacks x^T into
                # [kbp, ko, m] so the moving operand's DoubleRow pair stride
                # is small (1024B), matching the layout the fast x-stationary
                # path streams. Stationary wq slices are reused across m.
                m_shard = MT * 128
                MG = m_shard // 1024
                for mg in range(MG):
                    xq = xqp.tile([128, KB // 2, 2, 1024], FP8)
                    for kbp in range(KB // 2):
                        for ko in range(2):
                            nc.vector.tensor_scalar_mul(
                                xq[:, kbp, ko, :],
                                xt_f32[:, 2 * kbp + ko,
                                       mg * 1024:(mg + 1) * 1024],
                                xsc[:])
                    if emit_dsc is not None:
                        emit_dsc()
                        emit_dsc = None
                    ob = ostage.tile([128, 4, 1024], out_dt)
                    for nt in range(4):
                        po = opsum.tile([128, 2, 512], F32)
                        for kbp in range(KB // 2):
                            for j in range(2):
                                nc.tensor.matmul(
                                    po[:, j, :],
                                    wq[:, 2 * kbp:2 * kbp + 2,
                                       nt * 128:(nt + 1) * 128],
                                    xq[:, kbp, :, j * 512:(j + 1) * 512],
                                    start=(kbp == 0), stop=(kbp == KB // 2 - 1),
                                    perf_mode=mybir.MatmulPerfMode.DoubleRow,
                                )
                        dst = ob[:, nt:nt + 1, :].rearrange(
                            "p a (j m) -> p (a j) m", j=2)
                        nc.scalar.mul(dst, po[:], dsc[:])
                    nc.sync.dma_start(out=out_re[mg], in_=ob[:])
                return

            if wstat:
                # weight-stationary: out[n-part, m-free]; stationary = wq
                # slice (reused across m), moving = quantized x^T. Output is
                # n-major in DRAM; host transposes on reassembly.
                m_shard = MT * 128
                MG = m_shard // 1024
                for mg in range(MG):
                    xq = xqp.tile([128, KB, 1024], FP8)
                    nc.vector.tensor_scalar_mul(
                        xq[:], xt_f32[:, :, mg * 1024:(mg + 1) * 1024], xsc[:])
                    if emit_dsc is not None:
                        emit_dsc()
                        emit_dsc = None
                    ob = ostage.tile([128, 4, 1024], out_dt)
                    for nt in range(4):
                        po = opsum.tile([128, 2, 512], F32)
                        for kbp in range(KB // 2):
                            for j in range(2):
                                nc.tensor.matmul(
                                    po[:, j, :],
                                    wq[:, 2 * kbp:2 * kbp + 2,
                                       nt * 128:(nt + 1) * 128],
                                    xq[:, 2 * kbp:2 * kbp + 2,
                                       j * 512:(j + 1) * 512],
                                    start=(kbp == 0), stop=(kbp == KB // 2 - 1),
                                    perf_mode=mybir.MatmulPerfMode.DoubleRow,
                                )
                        dst = ob[:, nt:nt + 1, :].rearrange(
                            "p a (j m) -> p (a j) m", j=2)
                        gi = mg * 4 + nt
                        if deq_dve_mod and gi % deq_dve_mod == deq_dve_mod - 1:
                            nc.vector.tensor_scalar_mul(dst, po[:], dsc[:])
                        else:
                            nc.scalar.mul(dst, po[:], dsc[:])
                    nc.sync.dma_start(out=out_re[mg], in_=ob[:])
                return

            PSC = 2                      # m-tiles per PSUM out tile (2 banks)
            for c in range(SC):
                ob = ostage.tile([128, store_chunk, N], out_dt)
                for g in range(store_chunk // PSC):
                    po = opsum.tile([128, PSC, N], F32)
                    i0 = c * store_chunk + g * PSC
                    # quantize 2 m-tiles per DVE op (2x fp32 SBUF mode); the
                    # very first group quantizes per-m-tile so the first
                    # matmul starts half an op earlier
                    xq_t2 = xqp.tile([128, KB, PSC * 128], FP8)
                    if (q_split or fast_par) and c == 0 and g == 0:
                        for jq in range(PSC):
                            nc.vector.tensor_scalar_mul(
                                xq_t2[:, :, jq * 128:(jq + 1) * 128],
                                xt_f32[:, :, (i0 + jq) * 128:(i0 + jq + 1) * 128],
                                xsc[:],
                            )
                    else:
                        nc.vector.tensor_scalar_mul(
                            xq_t2[:], xt_f32[:, :, i0 * 128:(i0 + PSC) * 128],
                            xsc[:],
                        )
                    if emit_dsc is not None:
                        emit_dsc()
                        emit_dsc = None
                    if pb_mode == "quant":
                        continue
                    pm = (mybir.MatmulPerfMode.DoubleRowSwInterleave
                          if swi else mybir.MatmulPerfMode.DoubleRow)
                    if use_doublerow and kb_outer:
                        # kb-outer order: an accumulate into bank j never
                        # directly follows the matmul it accumulates onto
                        # (one other-bank matmul sits in between)
                        for kb in range(0, KB, 2):
                            for j in range(PSC):
                                xq_t = xq_t2[:, :, j * 128:(j + 1) * 128]
                                nc.tensor.matmul(
                                    po[:, j, :], xq_t[:, kb:kb + 2, :],
                                    wq[:, kb:kb + 2, :],
                                    start=(kb == 0), stop=(kb == KB - 2),
                                    perf_mode=pm,
                                )
                    elif use_doublerow:
                        for j in range(PSC):
                            xq_t = xq_t2[:, :, j * 128:(j + 1) * 128]
                            for kb in range(0, KB, 2):
                                nc.tensor.matmul(
                                    po[:, j, :], xq_t[:, kb:kb + 2, :],
                                    wq[:, kb:kb + 2, :],
                                    start=(kb == 0), stop=(kb == KB - 2),
                                    perf_mode=pm,
                                )
                    else:
                        for j in range(PSC):
                            xq_t = xq_t2[:, :, j * 128:(j + 1) * 128]
                            for kb in range(KB):
                                nc.tensor.matmul(
                                    po[:, j, :], xq_t[:, kb, :], wq[:, kb, :],
                                    start=(kb == 0), stop=(kb == KB - 1),
                                )
                    # dequant on ACT (activation Copy with scale AP); optionally
                    # route every deq_dve_mod-th group to DVE to split the load
                    gi = c * (store_chunk // PSC) + g
                    if deq_dve_mod and gi % deq_dve_mod == deq_dve_mod - 1:
                        nc.vector.tensor_scalar_mul(
                            ob[:, g * PSC:(g + 1) * PSC, :], po[:], dsc[:])
                    else:
                        nc.scalar.mul(ob[:, g * PSC:(g + 1) * PSC, :], po[:], dsc[:])
                if pb_mode == "full":
                    nc.sync.dma_start(out=out_re[c], in_=ob[:])


_CACHE: dict = {}


def _get_compiled(m_shard: int, **kw):
    key = (m_shard, tuple(sorted(kw.items())))
    if key not in _CACHE:
        _CACHE[key] = build_nc(m_shard, **kw)
    return _CACHE[key]


def run(x2d: np.ndarray, w: np.ndarray, trace: bool = False, **build_kw):
    """Run the SPMD kernel on [M, K] x and return ([M, N] out, BassKernelResults)."""
    M = x2d.shape[0]
    assert M % N_CORES == 0
    m_shard = M // N_CORES
    nc = _get_compiled(m_shard, **build_kw)
    shards = x2d.reshape(N_CORES, m_shard, K)
    w = np.ascontiguousarray(w, dtype=np.float32)
    in_maps = [
        {"x": np.ascontiguousarray(shards[c]), "w": w} for c in range(N_CORES)
    ]
    res = run_bass_kernel_spmd(nc, in_maps, core_ids=list(range(N_CORES)),
                               trace=trace)
    if build_kw.get("wstat"):
        out = np.concatenate(
            [res.results[c]["out"].T for c in range(N_CORES)], axis=0)
    else:
        out = np.concatenate(
            [res.results[c]["out"] for c in range(N_CORES)], axis=0)
    return out, res


def kernel(x: np.ndarray, weight: np.ndarray) -> np.ndarray:
    x = np.asarray(x, dtype=np.float32)
    weight = np.asarray(weight, dtype=np.float32)
    B, S, k = x.shape
    assert k == K
    out, _ = run(x.reshape(-1, K), weight)
    return out.reshape(B, S, N).astype(np.float32)



# revision 19
# speedup vs baseline: 5.3698x; 5.3698x over previous
"""Trainium2 Bass kernel for fp8-quantized dense matmul (dense_mlp).

Reference computation (per-tensor dynamic fp8 e4m3fn quantization):
    x:     [8, 8192, 512] f32  -> x2d [M=65536, K=512]
    w:     [512, 512] f32
    xs     = 448 / max(amax(|x|), 1e-12);  x_q = e4m3fn(x * xs)
    ws     = 448 / max(amax(|w|), 1e-12);  w_q = e4m3fn(w * ws)
    out    = (x_q @ w_q) * (1/xs) * (1/ws)          [M, 512] f32

Sharding: data-parallel over M across 8 cores (8192 rows each), weight
replicated; the x amax needs a cross-core AllReduce(max).

TRN2 fp8e4 (float8_e4m3) maxes out at +-240 (values in (240, 448] that OCP
e4m3fn can represent are Inf/NaN on TRN). We therefore quantize on-device
with scale' = 224/amax = (448/amax)/2. Scaling by an exact power of two
keeps every quantized value on the same relative grid (q' = q/2 exactly,
modulo the subnormal tail which is negligible), and the dequant factor
computed from the halved scales is exactly 4x the reference's factor,
cancelling the psum/4 -- so the result matches the reference bit-for-bit
up to f32 summation order (HW rel err 4e-7 in Normal matmul mode; the
default DoubleRow fp8 perf mode measures ~1e-4 from the PE's paired-
product accumulation precision, and is ~16% faster end-to-end).

Performance structure (measured on HW via repeat-loop slopes):
  phase A  ~60us: the 16.78MB x load is HBM-bound at ~278 GB/s/core (the
           practical ceiling with all 8 cores streaming; chunk size and
           DGE path don't move it). amax (DVE), f32 PE transposes and ACT
           psum->sbuf evac all hide under the load.
  AllReduce(max) of the x amax: ~10us floor, excluded from the loop
           metric and added as a constant; the weight load+quantize hides
           in this window.
  phase B  ~38us: PE-bound. Per DoubleRow matmul = LDWEIGHTS(~220cyc) +
           512cyc streaming; 128 MMs. Dequant (ACT, psum->fp16) and DVE
           quantize hide under PE; fp16 stores (rel err 2.3e-4, 87x
           inside the 2e-2 budget) fully hide -- f32 stores did not,
           which is the main win over the f32-out baseline (-10.5us).
Weight-stationary variants (3 layouts) measured 20+us SLOWER despite
stationary reuse; DoubleRowSwInterleave gives wrong results with the
standard AP layout. The output is produced m-major so no host transpose
is needed; the host casts fp16 -> f32 on reassembly.

xload="seq" (current default): partition p owns MT consecutive DRAM rows,
so every partition's loads (and stores, via the matching out view) walk a
single sequential HBM extent across chunks. Load-only probes measured
431 GB/s/core vs 278 for the old strided layout (62.8us -> 38.9us), and
store-only probes 10.1us vs 28.0us; head-to-head full-pipeline slope
(n=60, R=3/2003) improved ~6.5us. The m permutation introduced by the
load is exactly undone by the store view, so results are bit-identical
(rel err 2.340e-04 on HW, same as the old layout). Probes that did NOT
help here: scalar-ring stores (stall ACT, +40us on jmaj), deq_dve_mod=2
(DVE is 1x from PSUM, +11us), dma_chunk=8 (+25us), f32r transposes
(walrus codegen failure in this toolchain).

unroll (timing builds only): tile.For_i's back-edge is an all-engine
barrier + semaphore reset, serializing iterations. unroll=4 emits 4
pipeline bodies per trip (barrier amortized, point-to-point deps
between bodies): 91.5us -> 85.6us per body; unroll=8 no better
(cross-body overlap is capped by the shared xt_f32/amax_slots/scalar
tiles and pool buffer depths, not the barrier). kb_outer and
xld_bufs=6 measured neutral/worse on top. Remaining known headroom
(unimplemented): PE executes 256 f32 transposes (~30us) + 128 MMs
(~39-46us) per body in queue order -- quantize-before-transpose (fp8
transposes at 1cyc/row, 4x less evac/quantize traffic, exact numerics)
plus ACT/DVE drain rebalancing models out at ~60-65us/body but needs a
double-buffered x-transpose tile to overlap across bodies.
"""

from contextlib import nullcontext

import numpy as np

import concourse.bacc as bacc
import concourse.bass_isa as bass_isa
import concourse.mybir as mybir
import concourse.tile as tile
from concourse.bass_utils import run_bass_kernel_spmd
from concourse.masks import make_identity

F32 = mybir.dt.float32
F16 = mybir.dt.float16
FP8 = mybir.dt.float8e4

K = 512
N = 512
KB = K // 128  # k-blocks of 128 (partition-dim contraction tiles)
N_CORES = 8

# fp8 scale ceiling on TRN (e4m3 max normal is 240; 224 = 448/2 keeps the
# quantization grid exactly aligned with the reference's e4m3fn grid)
FP8_CEIL = 224.0


def build_nc(m_shard: int, n_cores: int = N_CORES, use_doublerow: bool = True,
             dma_chunk: int = 4, store_chunk: int = 2, repeat: int = 1,
             phase_a_only: bool = False, ostage_bufs: int | None = None,
             out_f16: bool = True, deq_dve_mod: int = 0, wstat: bool = False,
             empty_loop: bool = False, pa_mode: str = "full",
             pb_mode: str = "full", split_load: bool = False,
             swi: bool = False, load_pair: bool = False,
             kb_outer: bool = False, xld_bufs: int | None = None,
             small_xt: bool = False, swdge_load: bool = False,
             tr_f32r: bool = False, fast_par: bool = False,
             q_split: bool = True, pe_warm: int = 6,
             internal_io: bool = False, xload: str = "seq",
             store_eng: str = "sync", unroll: int = 1):
    """Build + compile the per-core SPMD program.

    m_shard: rows of x handled by this core (must be divisible by 128*dma_chunk)
    repeat: >1 builds a TIMING variant -- the x pipeline (phases A+B and the
        scale chain, minus the AllReduce, which cannot sit inside control
        flow) runs in a hardware For_i loop `repeat` times so per-iteration
        time can be resolved above the ~0.5ms axon dispatch noise.
    """
    MT = m_shard // 128          # number of 128-row m-tiles
    CH = MT // dma_chunk         # number of DMA chunks
    SC = MT // store_chunk       # number of store chunks

    nc = bacc.Bacc(
        trn_type="TRN2",
        target_bir_lowering=False,
        debug=False,
        num_devices=n_cores,
    )

    out_dt = F16 if out_f16 else F32
    # internal_io: timing-only builds keep the big x/out tensors in device
    # DRAM (garbage data; engine timing is data-independent) so per-call
    # host<->device transfers vanish -- the axon tunnel in this environment
    # is slow/noisy enough (~seconds per 100MB) to swamp the slope otherwise.
    if internal_io:
        x_in = nc.dram_tensor("xi", [m_shard, K], F32, kind="Internal")
    else:
        x_in = nc.dram_tensor("x", [m_shard, K], F32, kind="ExternalInput")
    w_in = nc.dram_tensor("w", [K, N], F32, kind="ExternalInput")
    # wstat: weight-stationary matmul writes the output n-major ([N, m]);
    # the host transposes back when reassembling shards
    out_shape = [N, m_shard] if wstat else [m_shard, N]
    if internal_io:
        out_d = nc.dram_tensor("oi", out_shape, out_dt, kind="Internal")
        out_tiny = nc.dram_tensor("out", [1, 1], F32, kind="ExternalOutput")
    else:
        out_d = nc.dram_tensor("out", out_shape, out_dt, kind="ExternalOutput")
        out_tiny = None

    # DRAM views:
    #  x rows (c*dma_chunk + j)*128 + p  ->  [c, p, j, k]
    if xload == "pmaj":
        # partition-contiguous: partition p reads dma_chunk consecutive rows
        # (8KB runs at dma_chunk=4) per chunk; m-row = c*512 + p*dc + j
        assert not load_pair and not wstat
        x_re = x_in.ap().rearrange("(c p j) k -> c p j k", p=128, j=dma_chunk)
        assert store_chunk == 2 and dma_chunk % 2 == 0
        out_re = out_d.ap().rearrange(
            "(cc p jp r) n -> (cc jp) p r n", p=128, jp=dma_chunk // 2, r=2)
    elif xload == "seq":
        # fully-sequential per partition: partition p owns MT consecutive
        # rows; chunk c continues p's extent (m-row = p*MT + c*dc + j), so
        # the whole 16.8MB load is 128 sequential streams
        assert not load_pair and not wstat
        x_re = x_in.ap().rearrange("(p c j) k -> c p j k", p=128, j=dma_chunk)
        assert store_chunk == 2
        out_re = out_d.ap().rearrange("(p c r) n -> c p r n", p=128, r=2)
    elif load_pair:
        # paired-row layout: partition p takes rows {.. + 2p, .. + 2p+1} so
        # each DRAM run is 4KB (2 consecutive rows) instead of 2KB; the m
        # permutation is absorbed by the load and store views
        assert store_chunk == 2
        x_re = x_in.ap().rearrange("(c j p r) k -> c p j r k",
                                   j=dma_chunk // 2, p=128, r=2)
    else:
        x_re = x_in.ap().rearrange("(c j p) k -> c p j k", j=dma_chunk, p=128)
    #  w rows kb*128 + p -> [p, kb, n]
    w_re = w_in.ap().rearrange("(kb p) n -> p kb n", p=128)
    if xload in ("pmaj", "seq"):
        pass  # out_re already set alongside the load view above
    elif wstat:
        # out_T[nt*128+p, mg*1024 + mm] -> [mg, p, nt, mm] (2KB runs per row)
        out_re = out_d.ap().rearrange("(nt p) (mg mm) -> mg p nt mm",
                                      p=128, mm=1024)
    elif load_pair:
        # row = c*256 + 2p + r; each partition writes 2 consecutive rows
        out_re = out_d.ap().rearrange("(c p r) n -> c p r n", p=128, r=2)
    else:
        out_re = out_d.ap().rearrange("(c j p) n -> c p j n", j=store_chunk, p=128)

    with tile.TileContext(nc) as tc:
        with (
            tc.tile_pool(name="pers", bufs=1) as pers,
            tc.tile_pool(name="xld",
                         bufs=xld_bufs if xld_bufs is not None
                         else (3 if wstat else max(2, 16 // dma_chunk))) as xld,
            tc.tile_pool(name="xqp", bufs=4 if wstat else 8) as xqp,
            tc.tile_pool(
                name="ostage",
                bufs=ostage_bufs if ostage_bufs is not None
                else (3 if wstat else (4 if store_chunk <= 2 else 3)),
            ) as ostage,
            tc.tile_pool(name="tpsum", bufs=2, space="PSUM") as tpsum,
            tc.tile_pool(name="opsum", bufs=2, space="PSUM") as opsum,
            tc.tile_pool(name="ccdram", bufs=1, space="DRAM") as ccdram,
        ):
            # ---------------- persistent tiles ----------------
            ident = pers.tile([128, 128], F32)
            w_f32 = pers.tile([128, KB, N], F32)
            wq = pers.tile([128, KB, N], FP8)
            # transposed x (K on partitions); probes shrink it to fit SBUF
            xt_f32 = pers.tile([128, KB, 128 if small_xt else m_shard], F32)
            amax_slots = pers.tile([128, CH - 1 + dma_chunk], F32)

            def sc(name):
                return pers.tile([128, 1], F32, name=name)

            wa_part, wa_bc, wa_c, wa_r = sc("wa_part"), sc("wa_bc"), sc("wa_c"), sc("wa_r")
            xa_part, xa_bc, xa_g, xa_c, xa_r = (
                sc("xa_part"), sc("xa_bc"), sc("xa_g"), sc("xa_c"), sc("xa_r"))
            xsc, inv_xsc, dsc = sc("xsc"), sc("inv_xsc"), sc("dsc")

            make_identity(nc, ident)

            ones_row = pers.tile([1, 128], F32, name="ones_row")
            xa_s = pers.tile([1, 1], F32, name="xa_s")
            if fast_par:
                nc.vector.memset(ones_row, 1.0)

            wpair = pers.tile([1, 2], F32, name="wpair")
            wsc_b = pers.tile([128, 2], F32, name="wsc_b")  # [wsc, 1/wsc] bcast

            def weight_path():
                # Quantize the (replicated) weight -- no collective needed.
                # Deliberately gpsimd-free: in the single-shot build this runs
                # during the x-amax AllReduce, and anything on gpsimd would
                # queue behind the collective's ~10us engine wait. The
                # partition reduce/broadcast go through PE instead.
                nc.sync.dma_start(out=w_f32[:], in_=w_re)
                nc.vector.tensor_reduce(
                    out=wa_part[:], in_=w_f32[:], axis=mybir.AxisListType.XY,
                    op=mybir.AluOpType.max, apply_absolute_value=True,
                )
                wa_t = tpsum.tile([1, 128], F32, name="wa_t", tag="tp")
                nc.tensor.transpose(wa_t[:], wa_part[:], ident[:])
                nc.vector.tensor_reduce(
                    out=wa_bc[0:1, :], in_=wa_t[:], axis=mybir.AxisListType.X,
                    op=mybir.AluOpType.max,
                )
                nc.vector.tensor_scalar_max(wa_c[0:1, :], wa_bc[0:1, :], 1e-12)
                # wsc = 224 * (1/wa)  (TT divide is not a valid TRN2 DVE op;
                # the extra rounding vs fl(224/wa) is <=1ulp on the scale)
                nc.vector.reciprocal(wa_r[0:1, :], wa_c[0:1, :])
                nc.vector.tensor_scalar_mul(wpair[:, 0:1], wa_r[0:1, :], FP8_CEIL)
                nc.vector.reciprocal(wpair[:, 1:2], wpair[:, 0:1])
                # broadcast [wsc, 1/wsc] to all 128 partitions: bounce the
                # 8B pair through DRAM, then re-read with a 0-stride
                # partition dim (exact; a PE-matmul broadcast would truncate
                # the scale to fp22; SBUF sources can't have 0-stride
                # partitions, DRAM sources can)
                wdram = ccdram.tile([1, 2], F32, name="wdram")
                nc.sync.dma_start(out=wdram[:], in_=wpair[:])
                nc.sync.dma_start(
                    out=wsc_b[:].rearrange("p (a b) -> p a b", a=1),
                    in_=wdram[:].partition_broadcast(128),
                )
                # quantize weight: wq = fp8(w * wsc)
                nc.scalar.mul(wq[:], w_f32[:], wsc_b[:, 0:1])

            # In timing builds the collective runs once, outside the loop
            # (collectives cannot appear inside control flow).
            timing_loop = repeat > 1
            if timing_loop:
                # w path cannot sit inside the loop (it must run once), and
                # instructions emitted after a For_i cannot execute within it
                weight_path()
                weight_path = None
            if timing_loop and n_cores > 1:
                nc.vector.memset(xa_bc, 1.0)
                cc_in0 = ccdram.tile([128, 1], F32)
                cc_out0 = ccdram.tile([128, 1], F32)
                nc.gpsimd.dma_start(out=cc_in0[:], in_=xa_bc[:])
                nc.gpsimd.collective_compute(
                    "AllReduce",
                    mybir.AluOpType.max,
                    replica_groups=[list(range(n_cores))],
                    ins=[cc_in0.opt()],
                    outs=[cc_out0.opt()],
                )
                nc.gpsimd.dma_start(out=xa_g[:], in_=cc_out0[:])

            if pa_mode == "none":
                # phase-B-only probe: stable zero inputs for the in-loop ops
                nc.vector.memset(xt_f32, 0.0)
                nc.vector.memset(amax_slots, 0.0)
            elif pa_mode in ("load", "amax"):
                nc.vector.memset(amax_slots, 0.0)

            # Manual unroll: U pipeline bodies per For_i trip. The For_i
            # back-edge is an ALL-ENGINE BARRIER + semaphore reset, which
            # serializes iterations; emitting U bodies per trip amortizes the
            # barrier and lets consecutive bodies overlap (body u+1's loads
            # under body u's matmuls) via point-to-point tile dependencies.
            if timing_loop:
                assert repeat % unroll == 0
                trip = repeat // unroll
            else:
                trip = repeat
                unroll = 1
            loop_cm = (
                tc.For_i(0, trip, 1, hint_engines=(mybir.EngineType.PE,))
                if timing_loop else nullcontext()
            )
            with loop_cm:
              for _u in range(unroll):
                x_pipeline(
                    nc, tc, CH, dma_chunk, SC, store_chunk, KB, MT, N,
                    x_re, out_re, xld, xqp, ostage, tpsum, opsum, ccdram,
                    ident, w_f32, wq, xt_f32, amax_slots,
                    xa_part, xa_bc, xa_g, xa_c, xa_r, xsc, inv_xsc, wsc_b, dsc,
                    n_cores, use_doublerow, use_collective=not timing_loop,
                    phase_a_only=phase_a_only, weight_path=weight_path,
                    out_dt=out_dt, deq_dve_mod=deq_dve_mod, wstat=wstat,
                    pa_mode=pa_mode, pb_mode=pb_mode, split_load=split_load,
                    empty_loop=empty_loop, swi=swi, load_pair=load_pair,
                    kb_outer=kb_outer, swdge_load=swdge_load, tr_f32r=tr_f32r,
                    fast_par=fast_par, ones_row=ones_row, xa_s=xa_s,
                    q_split=q_split, pe_warm=pe_warm, store_eng=store_eng,
                )

            if out_tiny is not None:
                nc.sync.dma_start(out=out_tiny.ap(), in_=xa_g[0:1, 0:1])

    nc.compile()
    return nc


def x_pipeline(nc, tc, CH, dma_chunk, SC, store_chunk, KB, MT, N,
               x_re, out_re, xld, xqp, ostage, tpsum, opsum, ccdram,
               ident, w_f32, wq, xt_f32, amax_slots,
               xa_part, xa_bc, xa_g, xa_c, xa_r, xsc, inv_xsc, wsc_b, dsc,
               n_cores, use_doublerow, use_collective, phase_a_only=False,
               weight_path=None, out_dt=F32, deq_dve_mod=0, wstat=False,
               pa_mode="full", pb_mode="full", split_load=False,
               empty_loop=False, swi=False, load_pair=False, kb_outer=False,
               swdge_load=False, tr_f32r=False, fast_par=False, ones_row=None,
               xa_s=None, q_split=False, pe_warm=0, store_eng="sync"):
            if empty_loop:
                nc.vector.memset(xa_part, 0.0)
                return
            # ---------------- phase A: load x, amax, transpose ----------------
            load_eng = (lambda c: nc.gpsimd if swdge_load
                        else (nc.scalar if (split_load and c % 2) else nc.sync))

            def load_dma(eng, xt, c):
                if load_pair:
                    eng.dma_start(
                        out=xt[:].rearrange("p (j r) k -> p j r k", r=2),
                        in_=x_re[c])
                else:
                    eng.dma_start(out=xt[:], in_=x_re[c])

            def load_view_j(c, j):
                return (x_re[c, :, j // 2, j % 2, :] if load_pair
                        else x_re[c, :, j, :])

            for c in range(CH if pa_mode != "none" else 0):
                xt = xld.tile([128, dma_chunk, K], F32)
                if c < CH - 1 or pa_mode == "load":
                    load_dma(load_eng(c), xt, c)
                    if pa_mode != "load":
                        nc.vector.tensor_reduce(
                            out=amax_slots[:, c:c + 1], in_=xt[:],
                            axis=mybir.AxisListType.XY,
                            op=mybir.AluOpType.max, apply_absolute_value=True,
                        )
                else:
                    # split the last chunk into per-m-tile DMAs + small amax
                    # ops so the final abs-max lands right after the final
                    # (small) load instead of one 2.2us op after a 1MiB one
                    for j in range(dma_chunk):
                        load_eng(c).dma_start(out=xt[:, j, :], in_=load_view_j(c, j))
                        nc.vector.tensor_reduce(
                            out=amax_slots[:, c + j:c + j + 1], in_=xt[:, j, :],
                            axis=mybir.AxisListType.X,
                            op=mybir.AluOpType.max, apply_absolute_value=True,
                        )
                if pa_mode in ("load", "amax"):
                    continue
                for j2 in range(dma_chunk // 2):
                    # two m-tiles per PSUM tile (2 banks) -> one FD-1024 evac
                    tp = tpsum.tile([128, 2, KB, 128], F32)
                    for j in (2 * j2, 2 * j2 + 1):
                        for kb in range(KB):
                            if tr_f32r:
                                # f32r transpose runs 1.5 cyc/row vs 2.0 for
                                # f32; pure data movement, bit-exact
                                F32R = mybir.dt.float32r
                                nc.tensor.transpose(
                                    tp[:, j % 2, kb, :].bitcast(F32R),
                                    xt[:, j, kb * 128:(kb + 1) * 128]
                                    .bitcast(F32R),
                                    ident[:].bitcast(F32R),
                                )
                            else:
                                nc.tensor.transpose(
                                    tp[:, j % 2, kb, :],
                                    xt[:, j, kb * 128:(kb + 1) * 128], ident[:],
                                )
                    i = c * dma_chunk + 2 * j2   # first of the 2 m-tiles
                    # evacuate transposed f32 tiles (ACT; PSUM -> SBUF)
                    # dest [128, kb, 2, 128] viewed per kb: [2 m-tiles, 128]
                    if pa_mode != "trans":
                        nc.scalar.copy(
                            out=xt_f32[:, :, i * 128:(i + 2) * 128]
                            .rearrange("p kb (j m) -> p j kb m", j=2),
                            in_=tp[:],
                        )

            if phase_a_only:
                return

            # ---------------- amax finalize + collective ----------------
            # Emission order matters: engine streams execute IN ORDER, so the
            # x-amax chain (which gates everything) is emitted before the
            # weight path on every engine it touches.
            nc.vector.tensor_reduce(
                out=xa_part[:], in_=amax_slots[:], axis=mybir.AxisListType.X,
                op=mybir.AluOpType.max,
            )
            if pe_warm:
                # keep the PE array busy (p-state) across the amax->scale
                # gap: dummy transposes gated on the last-loaded chunk, sized
                # to finish before the first quantize op can complete
                warm_src = (xt[:, dma_chunk - 1, 0:128] if pa_mode != "none"
                            else xt_f32[:, 0, 0:128])
                for wi in range(pe_warm):
                    tpw = tpsum.tile([128, 128], F32, name=f"pe_warm{wi}",
                                     tag="tp")
                    nc.tensor.transpose(tpw[:], warm_src, ident[:])
            if fast_par:
                # partition max + broadcast without gpsimd: PE transpose ->
                # DVE reduce (to SBUF) -> PE ones-matmul broadcast -> ACT evac
                xa_t = tpsum.tile([1, 128], F32, name="xa_t", tag="tp")
                nc.tensor.transpose(xa_t[:], xa_part[:], ident[:])
                nc.vector.tensor_reduce(
                    out=xa_s[:], in_=xa_t[:], axis=mybir.AxisListType.X,
                    op=mybir.AluOpType.max,
                )
                bc = tpsum.tile([128, 1], F32, name="xa_bc_ps", tag="tp")
                nc.tensor.matmul(bc[:], ones_row[0:1, :], xa_s[:],
                                 start=True, stop=True)
                nc.scalar.copy(xa_bc[:], bc[:])
            else:
                nc.gpsimd.partition_all_reduce(
                    xa_bc[:], xa_part[:], channels=128,
                    reduce_op=bass_isa.ReduceOp.max,
                )
            cc_in = cc_out = None
            if use_collective and n_cores > 1:
                cc_in = ccdram.tile([128, 1], F32)
                cc_out = ccdram.tile([128, 1], F32)
                # bounces on HWDGE (nc.sync, ~0.6us first-byte vs SWDGE
                # ~1-2us); SP-queue order stays hazard-free: loads ->
                # bounce-out -> w path -> bounce-in -> stores
                nc.sync.dma_start(out=cc_in[:], in_=xa_bc[:])
                nc.gpsimd.collective_compute(
                    "AllReduce",
                    mybir.AluOpType.max,
                    replica_groups=[list(range(n_cores))],
                    ins=[cc_in.opt()],
                    outs=[cc_out.opt()],
                )

            if weight_path is not None:
                # runs during the collective: the 1MB weight DMA + wq chain
                # fill the DMA/DVE/ACT gap instead of competing with phase A
                weight_path()

            if cc_out is not None:
                nc.sync.dma_start(out=xa_g[:], in_=cc_out[:])
            else:
                nc.vector.tensor_copy(xa_g[:], xa_bc[:])

            nc.vector.tensor_scalar_max(xa_c[:], xa_g[:], 1e-12)
            nc.vector.reciprocal(xa_r[:], xa_c[:])
            nc.vector.tensor_scalar_mul(xsc[:], xa_r[:], FP8_CEIL)
            def emit_dsc():
                # emitted after the first quantize: DVE executes in order, so
                # placing these two ops between xsc and quantize_0 would delay
                # the first matmul/store; dsc is only needed by the first
                # dequant, which waits on the matmuls anyway
                nc.vector.reciprocal(inv_xsc[:], xsc[:])
                nc.vector.tensor_tensor(
                    out=dsc[:], in0=inv_xsc[:], in1=wsc_b[:, 1:2],
                    op=mybir.AluOpType.mult,
                )

            # ---------------- phase B: quantize, matmul, dequant, store -------
            if wstat == 2:
                # weight-stationary, take 2: quantize repacks x^T into
                # [kbp, ko, m] so the moving operand's DoubleRow pair stride
                # is small (1024B), matching the layout the fast x-stationary
                # path streams. Stationary wq slices are reused across m.
                m_shard = MT * 128
                MG = m_shard // 1024
                for mg in range(MG):
                    xq = xqp.tile([128, KB // 2, 2, 1024], FP8)
                    for kbp in range(KB // 2):
                        for ko in range(2):
                            nc.vector.tensor_scalar_mul(
                                xq[:, kbp, ko, :],
                                xt_f32[:, 2 * kbp + ko,
                                       mg * 1024:(mg + 1) * 1024],
                                xsc[:])
                    if emit_dsc is not None:
                        emit_dsc()
                        emit_dsc = None
                    ob = ostage.tile([128, 4, 1024], out_dt)
                    for nt in range(4):
                        po = opsum.tile([128, 2, 512], F32)
                        for kbp in range(KB // 2):
                            for j in range(2):
                                nc.tensor.matmul(
                                    po[:, j, :],
                                    wq[:, 2 * kbp:2 * kbp + 2,
                                       nt * 128:(nt + 1) * 128],
                                    xq[:, kbp, :, j * 512:(j + 1) * 512],
                                    start=(kbp == 0), stop=(kbp == KB // 2 - 1),
                                    perf_mode=mybir.MatmulPerfMode.DoubleRow,
                                )
                        dst = ob[:, nt:nt + 1, :].rearrange(
                            "p a (j m) -> p (a j) m", j=2)
                        nc.scalar.mul(dst, po[:], dsc[:])
                    nc.sync.dma_start(out=out_re[mg], in_=ob[:])
                return

            if wstat:
                # weight-stationary: out[n-part, m-free]; stationary = wq
                # slice (reused across m), moving = quantized x^T. Output is
                # n-major in DRAM; host transposes on reassembly.
                m_shard = MT * 128
                MG = m_shard // 1024
                for mg in range(MG):
                    xq = xqp.tile([128, KB, 1024], FP8)
                    nc.vector.tensor_scalar_mul(
                        xq[:], xt_f32[:, :, mg * 1024:(mg + 1) * 1024], xsc[:])
                    if emit_dsc is not None:
                        emit_dsc()
                        emit_dsc = None
                    ob = ostage.tile([128, 4, 1024], out_dt)
                    for nt in range(4):
                        po = opsum.tile([128, 2, 512], F32)
                        for kbp in range(KB // 2):
                            for j in range(2):
                                nc.tensor.matmul(
                                    po[:, j, :],
                                    wq[:, 2 * kbp:2 * kbp + 2,
                                       nt * 128:(nt + 1) * 128],
                                    xq[:, 2 * kbp:2 * kbp + 2,
                                       j * 512:(j + 1) * 512],
                                    start=(kbp == 0), stop=(kbp == KB // 2 - 1),
                                    perf_mode=mybir.MatmulPerfMode.DoubleRow,
                                )
                        dst = ob[:, nt:nt + 1, :].rearrange(
                            "p a (j m) -> p (a j) m", j=2)
                        gi = mg * 4 + nt
                        if deq_dve_mod and gi % deq_dve_mod == deq_dve_mod - 1:
                            nc.vector.tensor_scalar_mul(dst, po[:], dsc[:])
                        else:
                            nc.scalar.mul(dst, po[:], dsc[:])
                    nc.sync.dma_start(out=out_re[mg], in_=ob[:])
                return

            if pb_mode == "store":
                # store-only probe: one memset staging tile, SC store DMAs
                ob_p = ostage.tile([128, store_chunk, N], out_dt)
                nc.vector.memset(ob_p, 0.0)
                for c in range(SC):
                    nc.sync.dma_start(out=out_re[c], in_=ob_p[:])
                return

            PSC = 2                      # m-tiles per PSUM out tile (2 banks)
            for c in range(SC):
                ob = ostage.tile([128, store_chunk, N], out_dt)
                for g in range(store_chunk // PSC):
                    po = opsum.tile([128, PSC, N], F32)
                    i0 = c * store_chunk + g * PSC
                    # quantize 2 m-tiles per DVE op (2x fp32 SBUF mode); the
                    # very first group quantizes per-m-tile so the first
                    # matmul starts half an op earlier
                    xq_t2 = xqp.tile([128, KB, PSC * 128], FP8)
                    if (q_split or fast_par) and c == 0 and g == 0:
                        for jq in range(PSC):
                            nc.vector.tensor_scalar_mul(
                                xq_t2[:, :, jq * 128:(jq + 1) * 128],
                                xt_f32[:, :, (i0 + jq) * 128:(i0 + jq + 1) * 128],
                                xsc[:],
                            )
                    else:
                        nc.vector.tensor_scalar_mul(
                            xq_t2[:], xt_f32[:, :, i0 * 128:(i0 + PSC) * 128],
                            xsc[:],
                        )
                    if emit_dsc is not None:
                        emit_dsc()
                        emit_dsc = None
                    if pb_mode == "quant":
                        continue
                    pm = (mybir.MatmulPerfMode.DoubleRowSwInterleave
                          if swi else mybir.MatmulPerfMode.DoubleRow)
                    if use_doublerow and kb_outer:
                        # kb-outer order: an accumulate into bank j never
                        # directly follows the matmul it accumulates onto
                        # (one other-bank matmul sits in between)
                        for kb in range(0, KB, 2):
                            for j in range(PSC):
                                xq_t = xq_t2[:, :, j * 128:(j + 1) * 128]
                                nc.tensor.matmul(
                                    po[:, j, :], xq_t[:, kb:kb + 2, :],
                                    wq[:, kb:kb + 2, :],
                                    start=(kb == 0), stop=(kb == KB - 2),
                                    perf_mode=pm,
                                )
                    elif use_doublerow:
                        for j in range(PSC):
                            xq_t = xq_t2[:, :, j * 128:(j + 1) * 128]
                            for kb in range(0, KB, 2):
                                nc.tensor.matmul(
                                    po[:, j, :], xq_t[:, kb:kb + 2, :],
                                    wq[:, kb:kb + 2, :],
                                    start=(kb == 0), stop=(kb == KB - 2),
                                    perf_mode=pm,
                                )
                    else:
                        for j in range(PSC):
                            xq_t = xq_t2[:, :, j * 128:(j + 1) * 128]
                            for kb in range(KB):
                                nc.tensor.matmul(
                                    po[:, j, :], xq_t[:, kb, :], wq[:, kb, :],
                                    start=(kb == 0), stop=(kb == KB - 1),
                                )
                    # dequant on ACT (activation Copy with scale AP); optionally
                    # route every deq_dve_mod-th group to DVE to split the load
                    gi = c * (store_chunk // PSC) + g
                    if deq_dve_mod and gi % deq_dve_mod == deq_dve_mod - 1:
                        nc.vector.tensor_scalar_mul(
                            ob[:, g * PSC:(g + 1) * PSC, :], po[:], dsc[:])
                    else:
                        nc.scalar.mul(ob[:, g * PSC:(g + 1) * PSC, :], po[:], dsc[:])
                if pb_mode == "full":
                    # stores on the scalar HWDGE ring (qActDynamicHW) decouple
                    # them from the loads' sync ring, so iteration i+1's loads
                    # aren't queued behind iteration i's stores
                    se = nc.scalar if store_eng == "scalar" else nc.sync
                    se.dma_start(out=out_re[c], in_=ob[:])


_CACHE: dict = {}


def _get_compiled(m_shard: int, **kw):
    key = (m_shard, tuple(sorted(kw.items())))
    if key not in _CACHE:
        _CACHE[key] = build_nc(m_shard, **kw)
    return _CACHE[key]


def run(x2d: np.ndarray, w: np.ndarray, trace: bool = False, **build_kw):
    """Run the SPMD kernel on [M, K] x and return ([M, N] out, BassKernelResults)."""
    M = x2d.shape[0]
    assert M % N_CORES == 0
    m_shard = M // N_CORES
    nc = _get_compiled(m_shard, **build_kw)
    shards = x2d.reshape(N_CORES, m_shard, K)
    w = np.ascontiguousarray(w, dtype=np.float32)
    in_maps = [
        {"x": np.ascontiguousarray(shards[c]), "w": w} for c in range(N_CORES)
    ]
    res = run_bass_kernel_spmd(nc, in_maps, core_ids=list(range(N_CORES)),
                               trace=trace)
    if build_kw.get("wstat"):
        out = np.concatenate(
            [res.results[c]["out"].T for c in range(N_CORES)], axis=0)
    else:
        out = np.concatenate(
            [res.results[c]["out"] for c in range(N_CORES)], axis=0)
    return out, res


def kernel(x: np.ndarray, weight: np.ndarray) -> np.ndarray:
    x = np.asarray(x, dtype=np.float32)
    weight = np.asarray(weight, dtype=np.float32)
    B, S, k = x.shape
    assert k == K
    out, _ = run(x.reshape(-1, K), weight)
    return out.reshape(B, S, N).astype(np.float32)

